# revision 19
# baseline (speedup 1.0000x reference)
"""Trainium2 Bass kernel for nn_LogicDense (difflogic dense layer).

Math (reference):
    w      = softmax(weight, axis=-1)            # [out_dim, 16]
    coeffs = w @ GATE_COEFFS                     # [out_dim, 4] = (c0, ca, cb, cab)
    a      = x[:, indices[0]]                    # [batch, out_dim]
    b      = x[:, indices[1]]
    out    = c0 + ca*a + cb*b + cab*a*b          # [batch, out_dim]

Strategy (8 NeuronCores, tensor-parallel over out_dim):
    - Host transposes x -> x_t [in_dim, batch] (fp16, replicated to all
      cores).
    - Core c owns output rows j in [2048*c, 2048*(c+1)).
    - Per 128-row chunk: one GPSIMD dma_gather pulls the 256 rows
      x_t[idx0[chunk]] ++ x_t[idx1[chunk]] from HBM into SBUF (row i of the
      index list lands on partition i%128, slot i//128; full batch on the
      free dim).
    - All 16 gates map [0,1]^2 -> [0,1] and softmax weights are convex, so
      out is in [0,1]. The output is therefore computed as
      out_u = 250*out + 2.5 (in [2.5, 252.5], safely inside u8) and the
      store DMA (SWDGE, gpsimd -- only SWDGE can cast) converts
      fp16 -> uint8 on the way to HBM, halving store traffic. The 250x
      scale folds into the gate-coefficient constants host-side (all four
      gcr rows x250; the +2.5 offset rides the c0 row because softmax
      weights sum to 1). Host decodes (u - 2.5)/250.
    - Per-partition coeff scalars give a 4-instruction combine:
         t = (250*cab)*b + 250*ca      (DVE tensor_scalar, fp16 4x mode)
         h = (250*cb)*b + (250*c0+2.5) (ACT Identity, scale/bias APs)
         o = t * a                     (DVE tensor_mul)
         o = o + h                     (DVE tensor_add)
    - Softmax+gate-coeff collapse is computed on device (ACT exp + DVE
      reduces, fp32) from the raw weight shard.
    - HBM/core: 32 MiB gather (fp16) + 8 MiB store (u8) = 40 MiB vs 48 in
      the all-fp16 version; the DMA roofline drops from ~140us to ~115us
      while DVE (~96us) and ACT (~60us) keep slack.
    - Decoupled rotating buffers (gather ab x4, t x2, h x3, out x6) with
      per-buffer rotating DMA semaphores keep gather DMA, ACT, DVE and
      store DMA all overlapped.
"""

import os
import sys

import numpy as np

sys.path.insert(0, "/opt/trn_rl_repo")

BATCH = 4096
IN_DIM = 8192
OUT_DIM = 16384
N_CORES = 8
J_SHARD = OUT_DIM // N_CORES        # 2048 output rows per core
CHUNK = 128                         # output rows per pipeline iteration
N_CHUNKS = J_SHARD // CHUNK         # 16
GCHUNK = 256                        # output rows per dma_gather call
N_GCH = J_SHARD // GCHUNK           # 8 gather iterations
NG = 4                              # gather buffer sets (a and b each)
NT = 2                              # t buffer sets (ts -> mul lifetime)
NH = 3                              # h buffer sets (h -> add lifetime)
NO = 3                              # paired output buffer sets
DVE_PRE = 14                        # DVE preamble (coeff) instruction count

OUT_SCALE = 250.0                   # out_u = OUT_SCALE*out + OUT_OFF
OUT_OFF = 2.5

GATE_COEFFS = np.array([
    [0, 0, 0, 0], [0, 0, 0, 1], [0, 1, 0, -1], [0, 1, 0, 0],
    [0, 0, 1, -1], [0, 0, 1, 0], [0, 1, 1, -2], [0, 1, 1, -1],
    [1, -1, -1, 1], [1, -1, -1, 2], [1, 0, -1, 0], [1, 0, -1, 1],
    [1, -1, 0, 0], [1, -1, 0, 1], [1, 0, 0, -1], [1, 0, 0, 0],
], dtype=np.float32)                # [16 gates, 4 bilinear coeffs]

_CACHE = {}
LAST_RESULT = None  # BassKernelResults of the most recent run (for profiling)


def _wrap_idx256(grp):
    """Wrap one 256-index gather list into dma_gather's index layout:
    index j lives at [j%16, j//16] of a [16, 16] block, replicated across
    the 8 groups of 16 partitions (the Q7 tx/rx cpus read the indices
    from different partition groups). Returns [128, 16] int16."""
    blk = grp.astype(np.int16).reshape(16, 16).T    # [16, 16]
    return np.tile(blk, (8, 1))                     # [128, 16]


def _wrap_core_idx(idx_pair):
    """Per-core [2, J_SHARD] -> [128, 32*N_GCH] int16; gather iter P uses
    cols [32P, 32P+16) for the a-gather (idx0 of outputs [256P, 256P+256))
    and [32P+16, 32P+32) for the b-gather (idx1)."""
    cols = []
    for c in range(N_GCH):
        s = slice(c * GCHUNK, (c + 1) * GCHUNK)
        cols.append(_wrap_idx256(idx_pair[0, s]))
        cols.append(_wrap_idx256(idx_pair[1, s]))
    return np.ascontiguousarray(np.concatenate(cols, axis=1))


def _build_program():
    import concourse.bacc as bacc
    import concourse.mybir as mybir
    from concourse.library_config import mlp
    from contextlib import ExitStack

    dt = mybir.dt
    AF = mybir.ActivationFunctionType

    nc = bacc.Bacc("TRN2", target_bir_lowering=False, debug=False)

    xt16 = nc.dram_tensor("xt16", [IN_DIM, BATCH], dt.float16,
                          kind="ExternalInput")
    xt8 = nc.dram_tensor("xt8", [IN_DIM, BATCH], dt.uint8,
                         kind="ExternalInput")
    idx = nc.dram_tensor("idx", [128, 32 * N_GCH], dt.int16,
                         kind="ExternalInput")
    wgt = nc.dram_tensor("wgt", [128, N_CHUNKS * 16], dt.float32,
                         kind="ExternalInput")
    gcr = nc.dram_tensor("gcr", [128, 4 * N_CHUNKS * 16], dt.float32,
                         kind="ExternalInput")
    out = nc.dram_tensor("out", [J_SHARD, BATCH], dt.uint8,
                         kind="ExternalOutput")

    W16 = N_CHUNKS * 16  # 256: free size of the wrapped weight / exp tiles

    with ExitStack() as ctx:
        sb = lambda name, shape, dty: ctx.enter_context(
            nc.sbuf_tensor(name, shape, dty))
        sb_idx = sb("sb_idx", [128, 32 * N_GCH], dt.int16)
        sb_w = sb("sb_w", [128, W16], dt.float32)
        sb_gc = sb("sb_gc", [128, 4 * W16], dt.float32)
        sb_e = sb("sb_e", [128, W16], dt.float32)
        sb_scr = sb("sb_scr", [128, W16], dt.float32)
        sb_s = sb("sb_s", [128, N_CHUNKS], dt.float32)
        sb_r = sb("sb_r", [128, N_CHUNKS], dt.float32)
        # coeff tile: [:, 16*k + c] = coeff k (0=c0,1=ca,2=cb,3=cab), chunk c
        sb_cc = sb("sb_cc", [128, 4 * N_CHUNKS], dt.float32)
        # gather dst: [128, 2, BATCH]; slot s holds compute chunk 2P+s
        a_bufs = [sb(f"a{k}", [128, 2, BATCH], dt.uint8) for k in range(NG)]
        b_bufs = [sb(f"b{k}", [128, 2, BATCH], dt.float16) for k in range(NG)]
        t_bufs = [sb(f"t{k}", [128, BATCH], dt.float16) for k in range(NT)]
        h_bufs = [sb(f"h{k}", [128, BATCH], dt.float16) for k in range(NH)]
        o_bufs = [sb(f"o{k}", [128, 2, BATCH], dt.float16) for k in range(NO)]

        # ts(i): t = (sa*cab) + 250*cb is ACT-shaped (scale/bias per
        # partition): even chunks compute it on ACT, odd on DVE (TS, u8 2x).
        # ACT stream per chunk: [t(i) (even i only), h(i)]:
        #   DVE: 8 ts (2.4us) + 16 mul + 16 add (2.3us)  ~= 92us
        #   ACT: 8 t (3.6us) + 16 h (3.6us) + exp        ~= 88us
        ts_on_act = lambda i: i % 2 == 0

        ops_act = []
        for i in range(N_CHUNKS):
            if ts_on_act(i):
                ops_act.append(('t', i))
            ops_act.append(('h', i))
        act_val = {op: n + 1 for n, op in enumerate(ops_act)}

        ops_dve = []  # DVE stream after the coeff preamble
        for i in range(N_CHUNKS):
            if not ts_on_act(i):
                ops_dve.append(('ts', i))
            if i > 0:
                ops_dve.append(('add', i - 1))
            ops_dve.append(('mul', i))
        ops_dve.append(('add', N_CHUNKS - 1))
        dve_val = {op: DVE_PRE + n + 1 for n, op in enumerate(ops_dve)}

        with (
            nc.Block() as block,
            nc.semaphore("s_pi") as s_pi,
            nc.semaphore("s_pw") as s_pw,
            nc.semaphore("s_pg") as s_pg,
            nc.semaphore("s_exp") as s_exp,
            nc.semaphore("s_ga0") as s_ga0,
            nc.semaphore("s_ga1") as s_ga1,
            nc.semaphore("s_ga2") as s_ga2,
            nc.semaphore("s_ga3") as s_ga3,
            nc.semaphore("s_gb0") as s_gb0,
            nc.semaphore("s_gb1") as s_gb1,
            nc.semaphore("s_gb2") as s_gb2,
            nc.semaphore("s_gb3") as s_gb3,
            nc.semaphore("s_st0") as s_st0,
            nc.semaphore("s_st1") as s_st1,
            nc.semaphore("s_st2") as s_st2,
            nc.semaphore("s_act") as s_act,
            nc.semaphore("s_dve") as s_dve,
        ):
            s_ga = [s_ga0, s_ga1, s_ga2, s_ga3]
            s_gb = [s_gb0, s_gb1, s_gb2, s_gb3]
            s_st = [s_st0, s_st1, s_st2]

            def cseg(k, i):  # per-partition scalar AP: coeff k, chunk i
                return sb_cc[:, 16 * k + i : 16 * k + i + 1]

            @block.sync
            def _(sync):
                sync.dma_start(sb_idx[:, :], idx[:, :]).then_inc(s_pi, 16)
                sync.dma_start(sb_w[:, :], wgt[:, :]).then_inc(s_pw, 16)
                sync.dma_start(sb_gc[:, :], gcr[:, :]).then_inc(s_pg, 16)

            @block.gpsimd
            def _(gp):
                gp.load_library(mlp)
                nreg = gp.alloc_register("nidx")
                gp.reg_mov(nreg, GCHUNK)
                gp.wait_ge(s_pi, 16)  # idx tile loaded

                def store(jp):  # store chunk pair jp (chunks 2jp, 2jp+1)
                    ko = jp % NO
                    gp.wait_ge(s_dve, dve_val[('add', 2 * jp + 1)])
                    if jp >= NO:
                        gp.wait_ge(s_st[ko], 16 * (jp // NO))
                    dst = out[jp * 2 * CHUNK:(jp + 1) * 2 * CHUNK, :]
                    gp.dma_start(dst.rearrange("(s p) f -> p s f", p=CHUNK),
                                 o_bufs[ko][:, :, :]).then_inc(s_st[ko], 16)

                next_store = 0

                for P in range(N_GCH):
                    kg = P % NG
                    last = 2 * (P - NG) + 1
                    if P >= NG:
                        # a[kg] free once h + ts of its last compute chunk
                        # done (mul(last) implies ts(last) and frees b[kg]).
                        gp.wait_ge(s_act, act_val[('h', last)])
                        gp.wait_ge(s_dve, dve_val[('mul', last)])
                        gp.wait_ge(s_ga[kg], 16 * (P // NG))
                    gp.dma_gather(
                        a_bufs[kg].ap(), xt8.ap(),
                        sb_idx[:, 32 * P:32 * P + 16], GCHUNK, nreg, BATCH,
                    ).then_inc(s_ga[kg], 16)
                    if P >= NG:
                        gp.wait_ge(s_gb[kg], 16 * (P // NG))
                    gp.dma_gather(
                        b_bufs[kg].ap(), xt16.ap(),
                        sb_idx[:, 32 * P + 16:32 * P + 32], GCHUNK, nreg,
                        BATCH,
                    ).then_inc(s_gb[kg], 16)
                    # one paired store per iter, lagging compute
                    if P >= 2:
                        for jp in range(next_store, P - 1):
                            store(jp)
                        next_store = P - 1
                for jp in range(next_store, N_CHUNKS // 2 - 1):
                    store(jp)
                # last pair split per chunk: overlap chunk-14's store with
                # the DVE add of chunk 15
                last_ko = (N_CHUNKS // 2 - 1) % NO
                for j in (N_CHUNKS - 2, N_CHUNKS - 1):
                    gp.wait_ge(s_dve, dve_val[('add', j)])
                    gp.dma_start(out[j * CHUNK:(j + 1) * CHUNK, :],
                                 o_bufs[last_ko][:, j % 2, :],
                                 ).then_inc(s_st[last_ko], 16)
                for ko in range(NO):
                    n_st = (N_CHUNKS // 2 - 1 - ko) // NO + 1
                    if ko == last_ko:
                        n_st += 1  # split final store added one extra inc
                    gp.wait_ge(s_st[ko], 16 * n_st)

            @block.scalar
            def _(sc):
                sc.wait_ge(s_pw, 16)
                sc.activation(sb_e[:, :], sb_w[:, :], AF.Exp).then_inc(s_exp, 1)
                sc.wait_ge(s_dve, DVE_PRE)  # coeff tile ready
                for kind, i in ops_act:
                    kg = (i // 2) % NG
                    sc.wait_ge(s_ga[kg], 16 * (i // 2 // NG + 1))
                    if kind == 't':
                        kt = i % NT
                        # t[kt] free once DVE mul of i-NT consumed it
                        if i >= NT:
                            sc.wait_ge(s_dve, dve_val[('mul', i - NT)])
                        # t = cab*sa + 250*cb  (u8 source read directly)
                        sc.activation(t_bufs[kt][:, :], a_bufs[kg][:, i % 2, :],
                                      AF.Identity,
                                      bias=cseg(2, i), scale=cseg(3, i),
                                      ).then_inc(s_act, 1)
                    else:
                        kh = i % NH
                        # h[kh] free once DVE add of i-NH completed
                        if i >= NH:
                            sc.wait_ge(s_dve, dve_val[('add', i - NH)])
                        # h = ca*sa + (250*c0 + 2.5)
                        sc.activation(h_bufs[kh][:, :], a_bufs[kg][:, i % 2, :],
                                      AF.Identity,
                                      bias=cseg(0, i), scale=cseg(1, i),
                                      ).then_inc(s_act, 1)

            @block.vector
            def _(v):
                X = mybir.AxisListType.X
                n = 0

                def step(ins):
                    nonlocal n
                    n += 1
                    ins.then_inc(s_dve, 1)

                v.wait_ge(s_exp, 1)
                v.wait_ge(s_pg, 16)  # gc tile loaded
                e3 = sb_e[:, :].rearrange("p (c g) -> p c g", g=16)
                step(v.reduce_sum(sb_s[:, :], e3, axis=X))
                v.wait_ge(s_dve, n)
                step(v.reciprocal(sb_r[:, :], sb_s[:, :]))
                for kk in range(4):
                    if kk > 0:
                        v.wait_ge(s_dve, n)  # scr free (prior reduce read it)
                    step(v.tensor_mul(sb_scr[:, :], sb_e[:, :],
                                      sb_gc[:, kk * W16:(kk + 1) * W16]))
                    v.wait_ge(s_dve, n)
                    step(v.reduce_sum(
                        sb_cc[:, 16 * kk:16 * (kk + 1)],
                        sb_scr[:, :].rearrange("p (c g) -> p c g", g=16),
                        axis=X))
                v.wait_ge(s_dve, n)  # all cc segments + r landed
                for kk in range(4):
                    step(v.tensor_mul(sb_cc[:, 16 * kk:16 * (kk + 1)],
                                      sb_cc[:, 16 * kk:16 * (kk + 1)],
                                      sb_r[:, :]))
                assert n == DVE_PRE
                v.wait_ge(s_dve, DVE_PRE)  # cc normalize landed
                MU, AD = mybir.AluOpType.mult, mybir.AluOpType.add
                for kind, i in ops_dve:
                    kt, kh = i % NT, i % NH
                    ko = (i // 2) % NO
                    kg = (i // 2) % NG
                    if kind == 'ts':
                        # t = (sa * cab) + 250*cb  (u8 src, 2x_2P mode)
                        v.wait_ge(s_ga[kg], 16 * (i // 2 // NG + 1))
                        if i >= NT:
                            # t[kt] free once mul of iter i-NT consumed it
                            v.wait_ge(s_dve, dve_val[('mul', i - NT)])
                        v.tensor_scalar(t_bufs[kt][:, :],
                                        a_bufs[kg][:, i % 2, :],
                                        cseg(3, i), cseg(2, i), MU, AD,
                                        ).then_inc(s_dve, 1)
                    elif kind == 'mul':
                        v.wait_ge(s_gb[kg], 16 * (i // 2 // NG + 1))
                        if ts_on_act(i):
                            v.wait_ge(s_act, act_val[('t', i)])
                        else:
                            v.wait_ge(s_dve, dve_val[('ts', i)])
                        if i // 2 >= NO:
                            # o[ko] free once store of pair i//2-NO completed
                            v.wait_ge(s_st[ko], 16 * (i // 2 // NO))
                        v.tensor_mul(o_bufs[ko][:, i % 2, :], t_bufs[kt][:, :],
                                     b_bufs[kg][:, i % 2, :]).then_inc(s_dve, 1)
                    else:  # add
                        v.wait_ge(s_act, act_val[('h', i)])
                        v.wait_ge(s_dve, dve_val[('mul', i)])
                        v.tensor_add(o_bufs[ko][:, i % 2, :],
                                     o_bufs[ko][:, i % 2, :],
                                     h_bufs[kh][:, :]).then_inc(s_dve, 1)

    nc.compile()
    return nc


def _get_program():
    if "nc" not in _CACHE:
        _CACHE["nc"] = _build_program()
    return _CACHE["nc"]


def kernel(x, weight, indices):
    global LAST_RESULT
    from concourse.bass_utils import run_bass_kernel_spmd

    x = np.asarray(x, dtype=np.float32)
    weight = np.asarray(weight, dtype=np.float32)
    indices = np.asarray(indices)

    nc = _get_program()

    xt = np.ascontiguousarray(x.T)                       # [in_dim, batch] f32
    xt16 = xt.astype(np.float16)
    xt8 = np.rint(xt * OUT_SCALE).astype(np.uint8)       # sa = 250*x in u8

    # gc replicate: c0 row carries the 250x output scale and +2.5 offset
    # (softmax weights sum to 1, so the offset rides the convex combination
    # exactly); cb row carries the 250x scale; ca/cab stay raw because the
    # gathered sa = 250*x already carries the factor.
    gc_scaled = GATE_COEFFS.copy()
    gc_scaled[:, 0] = OUT_SCALE * gc_scaled[:, 0] + OUT_OFF
    gc_scaled[:, 2] = OUT_SCALE * gc_scaled[:, 2]
    gc_rep = np.broadcast_to(
        gc_scaled.T.reshape(4, 1, 16),                   # [kk, 1, g]
        (4, N_CHUNKS, 16)).reshape(1, -1)
    gc_rep = np.ascontiguousarray(
        np.broadcast_to(gc_rep, (128, 4 * N_CHUNKS * 16)).astype(np.float32))

    in_maps = []
    for c in range(N_CORES):
        j0 = c * J_SHARD
        idx_c = _wrap_core_idx(indices[:, j0:j0 + J_SHARD])
        wsh = weight[j0:j0 + J_SHARD]                    # [2048, 16]
        w_wrapped = np.ascontiguousarray(
            wsh.reshape(N_CHUNKS, 128, 16).transpose(1, 0, 2)
            .reshape(128, N_CHUNKS * 16))
        in_maps.append({
            "xt16": xt16,
            "xt8": xt8,
            "idx": idx_c,
            "wgt": w_wrapped,
            "gcr": gc_rep,
        })

    trace = bool(os.environ.get("KERNEL_TRACE"))
    res = run_bass_kernel_spmd(nc, in_maps, core_ids=list(range(N_CORES)),
                               trace=trace)
    LAST_RESULT = res

    shards = [res.results[c]["out"] for c in range(N_CORES)]
    full = np.concatenate(shards, axis=0)                # [out_dim, batch] u8
    dec = (full.T.astype(np.float32) - OUT_OFF) * (1.0 / OUT_SCALE)
    return np.ascontiguousarray(dec)


# revision 20
# speedup vs baseline: 1.1510x; 1.1510x over previous
"""Trainium2 Bass kernel for nn_LogicDense (difflogic dense layer).

Math (reference):
    w      = softmax(weight, axis=-1)            # [out_dim, 16]
    coeffs = w @ GATE_COEFFS                     # [out_dim, 4] = (c0, ca, cb, cab)
    a      = x[:, indices[0]]                    # [batch, out_dim]
    b      = x[:, indices[1]]
    out    = c0 + ca*a + cb*b + cab*a*b          # [batch, out_dim]

Strategy (8 NeuronCores, tensor-parallel over out_dim):
    - Host transposes x -> x_t [in_dim, batch] (fp16, replicated to all
      cores).
    - Core c owns output rows j in [2048*c, 2048*(c+1)).
    - Per 128-row chunk: one GPSIMD dma_gather pulls the 256 rows
      x_t[idx0[chunk]] ++ x_t[idx1[chunk]] from HBM into SBUF (row i of the
      index list lands on partition i%128, slot i//128; full batch on the
      free dim).
    - All 16 gates map [0,1]^2 -> [0,1] and softmax weights are convex, so
      out is in [0,1]. The output is therefore computed as
      out_u = 250*out + 2.5 (in [2.5, 252.5], safely inside u8) and the
      store DMA (SWDGE, gpsimd -- only SWDGE can cast) converts
      fp16 -> uint8 on the way to HBM, halving store traffic. The 250x
      scale folds into the gate-coefficient constants host-side (all four
      gcr rows x250; the +2.5 offset rides the c0 row because softmax
      weights sum to 1). Host decodes (u - 2.5)/250.
    - Per-partition coeff scalars give a 4-instruction combine:
         t = (250*cab)*b + 250*ca      (DVE tensor_scalar, fp16 4x mode)
         h = (250*cb)*b + (250*c0+2.5) (ACT Identity, scale/bias APs)
         o = t * a                     (DVE tensor_mul)
         o = o + h                     (DVE tensor_add)
    - Softmax+gate-coeff collapse is computed on device (ACT exp + DVE
      reduces, fp32) from the raw weight shard.
    - HBM/core: 32 MiB gather (fp16) + 8 MiB store (u8) = 40 MiB vs 48 in
      the all-fp16 version; the DMA roofline drops from ~140us to ~115us
      while DVE (~96us) and ACT (~60us) keep slack.
    - Decoupled rotating buffers (gather ab x4, t x2, h x3, out x6) with
      per-buffer rotating DMA semaphores keep gather DMA, ACT, DVE and
      store DMA all overlapped.
"""

import os
import sys

import numpy as np

sys.path.insert(0, "/opt/trn_rl_repo")

BATCH = 4096
IN_DIM = 8192
OUT_DIM = 16384
N_CORES = 8
J_SHARD = OUT_DIM // N_CORES        # 2048 output rows per core
CHUNK = 128                         # output rows per pipeline iteration
N_CHUNKS = J_SHARD // CHUNK         # 16
GCHUNK = 256                        # output rows per dma_gather call
N_GCH = J_SHARD // GCHUNK           # 8 gather iterations
NG = 4                              # gather buffer sets (a and b each)
NT = 2                              # t buffer sets (ts -> mul lifetime)
NH = 3                              # h buffer sets (h -> add lifetime)
NO = 3                              # paired output buffer sets
DVE_PRE = 14                        # DVE preamble (coeff) instruction count

OUT_SCALE = 250.0                   # out_u = OUT_SCALE*out + OUT_OFF
OUT_OFF = 2.5

GATE_COEFFS = np.array([
    [0, 0, 0, 0], [0, 0, 0, 1], [0, 1, 0, -1], [0, 1, 0, 0],
    [0, 0, 1, -1], [0, 0, 1, 0], [0, 1, 1, -2], [0, 1, 1, -1],
    [1, -1, -1, 1], [1, -1, -1, 2], [1, 0, -1, 0], [1, 0, -1, 1],
    [1, -1, 0, 0], [1, -1, 0, 1], [1, 0, 0, -1], [1, 0, 0, 0],
], dtype=np.float32)                # [16 gates, 4 bilinear coeffs]

_CACHE = {}
LAST_RESULT = None  # BassKernelResults of the most recent run (for profiling)


def _wrap_idx256(grp):
    """Wrap one 256-index gather list into dma_gather's index layout:
    index j lives at [j%16, j//16] of a [16, 16] block, replicated across
    the 8 groups of 16 partitions (the Q7 tx/rx cpus read the indices
    from different partition groups). Returns [128, 16] int16."""
    blk = grp.astype(np.int16).reshape(16, 16).T    # [16, 16]
    return np.tile(blk, (8, 1))                     # [128, 16]


def _wrap_core_idx(idx_pair):
    """Per-core [2, J_SHARD] -> [128, 32*N_GCH] int16; gather iter P uses
    cols [32P, 32P+16) for the a-gather (idx0 of outputs [256P, 256P+256))
    and [32P+16, 32P+32) for the b-gather (idx1)."""
    cols = []
    for c in range(N_GCH):
        s = slice(c * GCHUNK, (c + 1) * GCHUNK)
        cols.append(_wrap_idx256(idx_pair[0, s]))
        cols.append(_wrap_idx256(idx_pair[1, s]))
    return np.ascontiguousarray(np.concatenate(cols, axis=1))


def _build_program():
    import concourse.bacc as bacc
    import concourse.mybir as mybir
    from concourse.library_config import mlp
    from contextlib import ExitStack

    dt = mybir.dt
    AF = mybir.ActivationFunctionType

    nc = bacc.Bacc("TRN2", target_bir_lowering=False, debug=False)

    xt16 = nc.dram_tensor("xt16", [IN_DIM, BATCH], dt.float16,
                          kind="ExternalInput")
    xt8 = nc.dram_tensor("xt8", [IN_DIM, BATCH], dt.uint8,
                         kind="ExternalInput")
    idx = nc.dram_tensor("idx", [128, 32 * N_GCH], dt.int16,
                         kind="ExternalInput")
    wgt = nc.dram_tensor("wgt", [128, N_CHUNKS * 16], dt.float32,
                         kind="ExternalInput")
    gcr = nc.dram_tensor("gcr", [128, 4 * N_CHUNKS * 16], dt.float32,
                         kind="ExternalInput")
    out = nc.dram_tensor("out", [J_SHARD, BATCH], dt.uint8,
                         kind="ExternalOutput")

    W16 = N_CHUNKS * 16  # 256: free size of the wrapped weight / exp tiles

    with ExitStack() as ctx:
        sb = lambda name, shape, dty: ctx.enter_context(
            nc.sbuf_tensor(name, shape, dty))
        sb_idx = sb("sb_idx", [128, 32 * N_GCH], dt.int16)
        sb_w = sb("sb_w", [128, W16], dt.float32)
        sb_gc = sb("sb_gc", [128, 4 * W16], dt.float32)
        sb_e = sb("sb_e", [128, W16], dt.float32)
        sb_scr = sb("sb_scr", [128, W16], dt.float32)
        sb_s = sb("sb_s", [128, N_CHUNKS], dt.float32)
        sb_r = sb("sb_r", [128, N_CHUNKS], dt.float32)
        # coeff tile: [:, 16*k + c] = coeff k (0=c0,1=ca,2=cb,3=cab), chunk c
        sb_cc = sb("sb_cc", [128, 4 * N_CHUNKS], dt.float32)
        # gather dst: [128, 2, BATCH]; slot s holds compute chunk 2P+s
        a_bufs = [sb(f"a{k}", [128, 2, BATCH], dt.uint8) for k in range(NG)]
        b_bufs = [sb(f"b{k}", [128, 2, BATCH], dt.float16) for k in range(NG)]
        t_bufs = [sb(f"t{k}", [128, BATCH], dt.float16) for k in range(NT)]
        h_bufs = [sb(f"h{k}", [128, BATCH], dt.float16) for k in range(NH)]
        o_bufs = [sb(f"o{k}", [128, 2, BATCH], dt.float16) for k in range(NO)]

        # ts(i): t = (sa*cab) + 250*cb is ACT-shaped (scale/bias per
        # partition): even chunks compute it on ACT, odd on DVE (TS, u8 2x).
        # ACT stream per chunk: [t(i) (even i only), h(i)]:
        #   DVE: 8 ts (2.4us) + 16 mul + 16 add (2.3us)  ~= 92us
        #   ACT: 8 t (3.6us) + 16 h (3.6us) + exp        ~= 88us
        ts_on_act = lambda i: i % 2 == 0

        ops_act = []
        for i in range(N_CHUNKS):
            if ts_on_act(i):
                ops_act.append(('t', i))
            ops_act.append(('h', i))
        act_val = {op: n + 1 for n, op in enumerate(ops_act)}

        ops_dve = []  # DVE stream after the coeff preamble
        for i in range(N_CHUNKS):
            if not ts_on_act(i):
                ops_dve.append(('ts', i))
            if i > 0:
                ops_dve.append(('add', i - 1))
            ops_dve.append(('mul', i))
        ops_dve.append(('add', N_CHUNKS - 1))
        dve_val = {op: DVE_PRE + n + 1 for n, op in enumerate(ops_dve)}

        with (
            nc.Block() as block,
            nc.semaphore("s_pi") as s_pi,
            nc.semaphore("s_pw") as s_pw,
            nc.semaphore("s_pg") as s_pg,
            nc.semaphore("s_exp") as s_exp,
            nc.semaphore("s_ga0") as s_ga0,
            nc.semaphore("s_ga1") as s_ga1,
            nc.semaphore("s_ga2") as s_ga2,
            nc.semaphore("s_ga3") as s_ga3,
            nc.semaphore("s_gb0") as s_gb0,
            nc.semaphore("s_gb1") as s_gb1,
            nc.semaphore("s_gb2") as s_gb2,
            nc.semaphore("s_gb3") as s_gb3,
            nc.semaphore("s_st0") as s_st0,
            nc.semaphore("s_st1") as s_st1,
            nc.semaphore("s_st2") as s_st2,
            nc.semaphore("s_act") as s_act,
            nc.semaphore("s_dve") as s_dve,
        ):
            s_ga = [s_ga0, s_ga1, s_ga2, s_ga3]
            s_gb = [s_gb0, s_gb1, s_gb2, s_gb3]
            s_st = [s_st0, s_st1, s_st2]

            def cseg(k, i):  # per-partition scalar AP: coeff k, chunk i
                return sb_cc[:, 16 * k + i : 16 * k + i + 1]

            @block.sync
            def _(sync):
                sync.dma_start(sb_idx[:, :], idx[:, :]).then_inc(s_pi, 16)
                sync.dma_start(sb_w[:, :], wgt[:, :]).then_inc(s_pw, 16)
                sync.dma_start(sb_gc[:, :], gcr[:, :]).then_inc(s_pg, 16)

            @block.gpsimd
            def _(gp):
                gp.load_library(mlp)
                nreg = gp.alloc_register("nidx")
                gp.reg_mov(nreg, GCHUNK)
                gp.wait_ge(s_pi, 16)  # idx tile loaded

                def store(jp):  # store chunk pair jp (chunks 2jp, 2jp+1)
                    ko = jp % NO
                    gp.wait_ge(s_dve, dve_val[('add', 2 * jp + 1)])
                    if jp >= NO:
                        gp.wait_ge(s_st[ko], 16 * (jp // NO))
                    dst = out[jp * 2 * CHUNK:(jp + 1) * 2 * CHUNK, :]
                    gp.dma_start(dst.rearrange("(s p) f -> p s f", p=CHUNK),
                                 o_bufs[ko][:, :, :]).then_inc(s_st[ko], 16)

                next_store = 0

                for P in range(N_GCH):
                    kg = P % NG
                    last = 2 * (P - NG) + 1
                    if P >= NG:
                        # a[kg] free once h + ts of its last compute chunk
                        # done (mul(last) implies ts(last) and frees b[kg]).
                        gp.wait_ge(s_act, act_val[('h', last)])
                        gp.wait_ge(s_dve, dve_val[('mul', last)])
                        gp.wait_ge(s_ga[kg], 16 * (P // NG))
                    gp.dma_gather(
                        a_bufs[kg].ap(), xt8.ap(),
                        sb_idx[:, 32 * P:32 * P + 16], GCHUNK, nreg, BATCH,
                    ).then_inc(s_ga[kg], 16)
                    if P >= NG:
                        gp.wait_ge(s_gb[kg], 16 * (P // NG))
                    gp.dma_gather(
                        b_bufs[kg].ap(), xt16.ap(),
                        sb_idx[:, 32 * P + 16:32 * P + 32], GCHUNK, nreg,
                        BATCH,
                    ).then_inc(s_gb[kg], 16)
                    # one paired store per iter, lagging compute
                    if P >= 2:
                        for jp in range(next_store, P - 1):
                            store(jp)
                        next_store = P - 1
                for jp in range(next_store, N_CHUNKS // 2):
                    store(jp)
                for ko in range(NO):
                    n_st = (N_CHUNKS // 2 - 1 - ko) // NO + 1
                    gp.wait_ge(s_st[ko], 16 * n_st)

            @block.scalar
            def _(sc):
                sc.wait_ge(s_pw, 16)
                sc.activation(sb_e[:, :], sb_w[:, :], AF.Exp).then_inc(s_exp, 1)
                sc.wait_ge(s_dve, DVE_PRE)  # coeff tile ready
                for kind, i in ops_act:
                    kg = (i // 2) % NG
                    sc.wait_ge(s_ga[kg], 16 * (i // 2 // NG + 1))
                    if kind == 't':
                        kt = i % NT
                        # t[kt] free once DVE mul of i-NT consumed it
                        if i >= NT:
                            sc.wait_ge(s_dve, dve_val[('mul', i - NT)])
                        # t = cab*sa + 250*cb  (u8 source read directly)
                        sc.activation(t_bufs[kt][:, :], a_bufs[kg][:, i % 2, :],
                                      AF.Identity,
                                      bias=cseg(2, i), scale=cseg(3, i),
                                      ).then_inc(s_act, 1)
                    else:
                        kh = i % NH
                        # h[kh] free once DVE add of i-NH completed
                        if i >= NH:
                            sc.wait_ge(s_dve, dve_val[('add', i - NH)])
                        # h = ca*sa + (250*c0 + 2.5)
                        sc.activation(h_bufs[kh][:, :], a_bufs[kg][:, i % 2, :],
                                      AF.Identity,
                                      bias=cseg(0, i), scale=cseg(1, i),
                                      ).then_inc(s_act, 1)

            @block.vector
            def _(v):
                X = mybir.AxisListType.X
                n = 0

                def step(ins):
                    nonlocal n
                    n += 1
                    ins.then_inc(s_dve, 1)

                v.wait_ge(s_exp, 1)
                v.wait_ge(s_pg, 16)  # gc tile loaded
                e3 = sb_e[:, :].rearrange("p (c g) -> p c g", g=16)
                step(v.reduce_sum(sb_s[:, :], e3, axis=X))
                v.wait_ge(s_dve, n)
                step(v.reciprocal(sb_r[:, :], sb_s[:, :]))
                for kk in range(4):
                    if kk > 0:
                        v.wait_ge(s_dve, n)  # scr free (prior reduce read it)
                    step(v.tensor_mul(sb_scr[:, :], sb_e[:, :],
                                      sb_gc[:, kk * W16:(kk + 1) * W16]))
                    v.wait_ge(s_dve, n)
                    step(v.reduce_sum(
                        sb_cc[:, 16 * kk:16 * (kk + 1)],
                        sb_scr[:, :].rearrange("p (c g) -> p c g", g=16),
                        axis=X))
                v.wait_ge(s_dve, n)  # all cc segments + r landed
                for kk in range(4):
                    step(v.tensor_mul(sb_cc[:, 16 * kk:16 * (kk + 1)],
                                      sb_cc[:, 16 * kk:16 * (kk + 1)],
                                      sb_r[:, :]))
                assert n == DVE_PRE
                v.wait_ge(s_dve, DVE_PRE)  # cc normalize landed
                MU, AD = mybir.AluOpType.mult, mybir.AluOpType.add
                for kind, i in ops_dve:
                    kt, kh = i % NT, i % NH
                    ko = (i // 2) % NO
                    kg = (i // 2) % NG
                    if kind == 'ts':
                        # t = (sa * cab) + 250*cb  (u8 src, 2x_2P mode)
                        v.wait_ge(s_ga[kg], 16 * (i // 2 // NG + 1))
                        if i >= NT:
                            # t[kt] free once mul of iter i-NT consumed it
                            v.wait_ge(s_dve, dve_val[('mul', i - NT)])
                        v.tensor_scalar(t_bufs[kt][:, :],
                                        a_bufs[kg][:, i % 2, :],
                                        cseg(3, i), cseg(2, i), MU, AD,
                                        ).then_inc(s_dve, 1)
                    elif kind == 'mul':
                        v.wait_ge(s_gb[kg], 16 * (i // 2 // NG + 1))
                        if ts_on_act(i):
                            v.wait_ge(s_act, act_val[('t', i)])
                        else:
                            v.wait_ge(s_dve, dve_val[('ts', i)])
                        if i // 2 >= NO:
                            # o[ko] free once store of pair i//2-NO completed
                            v.wait_ge(s_st[ko], 16 * (i // 2 // NO))
                        v.tensor_mul(o_bufs[ko][:, i % 2, :], t_bufs[kt][:, :],
                                     b_bufs[kg][:, i % 2, :]).then_inc(s_dve, 1)
                    else:  # add
                        v.wait_ge(s_act, act_val[('h', i)])
                        v.wait_ge(s_dve, dve_val[('mul', i)])
                        v.tensor_add(o_bufs[ko][:, i % 2, :],
                                     o_bufs[ko][:, i % 2, :],
                                     h_bufs[kh][:, :]).then_inc(s_dve, 1)

    nc.compile()
    return nc


def _get_program():
    if "nc" not in _CACHE:
        _CACHE["nc"] = _build_program()
    return _CACHE["nc"]


def kernel(x, weight, indices):
    global LAST_RESULT
    from concourse.bass_utils import run_bass_kernel_spmd

    x = np.asarray(x, dtype=np.float32)
    weight = np.asarray(weight, dtype=np.float32)
    indices = np.asarray(indices)

    nc = _get_program()

    xt = np.ascontiguousarray(x.T)                       # [in_dim, batch] f32
    xt16 = xt.astype(np.float16)
    xt8 = np.rint(xt * OUT_SCALE).astype(np.uint8)       # sa = 250*x in u8

    # gc replicate: c0 row carries the 250x output scale and +2.5 offset
    # (softmax weights sum to 1, so the offset rides the convex combination
    # exactly); cb row carries the 250x scale; ca/cab stay raw because the
    # gathered sa = 250*x already carries the factor.
    gc_scaled = GATE_COEFFS.copy()
    gc_scaled[:, 0] = OUT_SCALE * gc_scaled[:, 0] + OUT_OFF
    gc_scaled[:, 2] = OUT_SCALE * gc_scaled[:, 2]
    gc_rep = np.broadcast_to(
        gc_scaled.T.reshape(4, 1, 16),                   # [kk, 1, g]
        (4, N_CHUNKS, 16)).reshape(1, -1)
    gc_rep = np.ascontiguousarray(
        np.broadcast_to(gc_rep, (128, 4 * N_CHUNKS * 16)).astype(np.float32))

    in_maps = []
    for c in range(N_CORES):
        j0 = c * J_SHARD
        idx_c = _wrap_core_idx(indices[:, j0:j0 + J_SHARD])
        wsh = weight[j0:j0 + J_SHARD]                    # [2048, 16]
        w_wrapped = np.ascontiguousarray(
            wsh.reshape(N_CHUNKS, 128, 16).transpose(1, 0, 2)
            .reshape(128, N_CHUNKS * 16))
        in_maps.append({
            "xt16": xt16,
            "xt8": xt8,
            "idx": idx_c,
            "wgt": w_wrapped,
            "gcr": gc_rep,
        })

    trace = bool(os.environ.get("KERNEL_TRACE"))
    res = run_bass_kernel_spmd(nc, in_maps, core_ids=list(range(N_CORES)),
                               trace=trace)
    LAST_RESULT = res

    shards = [res.results[c]["out"] for c in range(N_CORES)]
    full = np.concatenate(shards, axis=0)                # [out_dim, batch] u8
    dec = (full.T.astype(np.float32) - OUT_OFF) * (1.0 / OUT_SCALE)
    return np.ascontiguousarray(dec)
